# revision 9
# baseline (speedup 1.0000x reference)
"""CrossCompressUnit kernel for TRN2, 8 NeuronCores, batch-sharded data parallel.

Math (per row b):
  v_out[b,:] = v[b,:]*(e[b].w_vv) + e[b,:]*(v[b].w_ev) + (b_vv+b_ev)
  e_out[b,:] = v[b,:]*(e[b].w_ve) + e[b,:]*(v[b].w_ee) + (b_ve+b_ee)

v4 design (bf16 end-to-end; DMA floor ~93us/core at 33.6MB traffic):
  - Host converts v,e to bf16; device reads/writes bf16 HBM; host upcasts
    outputs to f32 (rel-err ~7e-3 << 2e-2 gate).
  - All row-dots on PE: per [128,256] sub-tile, 4 transposes (bf16 LDW+MM
    with FWL) + 4 dot matmuls (vT/eT halves x [128,(2,4)] w4, f32 PSUM
    accumulation over the two 128-dim halves).
  - Work fused at QUAD granularity (4 sub-tiles, [128,1024]) to amortize
    the ~180-280ns/instr DVE/ACT overheads: one PSUM->SBUF copy per input
    per quad (ACT: vT, DVE: eT), one [128,32] s-copy (ACT), one final add
    per output per quad (DVE tensor_tensor, 2x), one Pool broadcast
    multiply per quad for u_e.
  - All DVE operands bf16 (incl. s scalars and bias) to enable the DVE
    2x/4x perf modes, which hardware only grants when every operand is
    2-byte.
  - Loads on sync (HWDGE SP ring), stores on scalar (HWDGE ACT ring),
    1MB DMAs ([128,4096] bf16 mega-tiles).
"""

import sys

sys.path.insert(0, "/opt/trn_rl_repo")

import numpy as np

import concourse.bass as bass
import concourse.bacc as bacc_mod
import concourse.mybir as mybir
from concourse.bass_utils import run_bass_kernel_spmd
from concourse.tile import TileContext

N_CORES = 8
B_FULL = 131072
DIM = 256
B_CORE = B_FULL // N_CORES  # 16384
P = 128

MEGA_ROWS = 2048                  # rows per mega-tile -> [128,4096] bf16 = 1MB DMA
ROWS_PER_PART = MEGA_ROWS // P    # 16 sub-tiles per mega-tile
N_MEGA = B_CORE // MEGA_ROWS      # 8
QUAD = 4                          # sub-tiles fused per quad unit
N_QUADS = ROWS_PER_PART // QUAD   # 4 quads per mega-tile

F32 = mybir.dt.float32
BF16 = mybir.dt.bfloat16
AluOp = mybir.AluOpType
ActFn = mybir.ActivationFunctionType

_COMPILED = {}


def build_program():
    nc = bacc_mod.Bacc()

    v_d = nc.declare_dram_parameter("v", [B_CORE, DIM], BF16, isOutput=False)
    e_d = nc.declare_dram_parameter("e", [B_CORE, DIM], BF16, isOutput=False)
    w4_d = nc.declare_dram_parameter("w4", [DIM, 4], BF16, isOutput=False)
    ident_d = nc.declare_dram_parameter("ident", [P, P], BF16, isOutput=False)
    bias_d = nc.declare_dram_parameter("bias", [P, 2], F32, isOutput=False)
    vout_d = nc.declare_dram_parameter("vout", [B_CORE, DIM], BF16, isOutput=True)
    eout_d = nc.declare_dram_parameter("eout", [B_CORE, DIM], BF16, isOutput=True)

    D4 = QUAD * DIM  # 1024, one quad

    with TileContext(nc) as tc:
        with (
            tc.tile_pool(name="consts", bufs=1) as consts,
            tc.tile_pool(name="vin", bufs=3) as vin_pool,
            tc.tile_pool(name="ein", bufs=3) as ein_pool,
            tc.tile_pool(name="vo", bufs=3) as vo_pool,
            tc.tile_pool(name="eo", bufs=3) as eo_pool,
            tc.tile_pool(name="tsb", bufs=2) as tsb_pool,
            tc.tile_pool(name="ssb", bufs=3) as ssb_pool,
            tc.tile_pool(name="ut", bufs=2) as ut_pool,
            tc.tile_pool(name="tps", bufs=2, space=bass.MemorySpace.PSUM) as tps_pool,
            tc.tile_pool(name="sps", bufs=2, space=bass.MemorySpace.PSUM) as sps_pool,
        ):
            # --- constants ---
            identity = consts.tile([P, P], BF16)
            nc.sync.dma_start(out=identity[:], in_=ident_d[:])
            w4_sb = consts.tile([P, 2, 4], BF16)  # [dim_in_half, half, 4]
            nc.sync.dma_start(
                out=w4_sb[:], in_=w4_d.rearrange("(h p) w -> p h w", h=2)
            )
            bias_sb = consts.tile([P, 2], F32)
            nc.sync.dma_start(out=bias_sb[:], in_=bias_d[:])

            out_tiles = []
            pend = None  # quad awaiting dots+elementwise (1-quad software skew)

            def emit_quad_head(v_sb, e_sb, vo_sb, eo_sb, q):
                """transposes + PSUM->SBUF copies for quad q of this mega."""
                c0 = q * D4
                vT_ps = tps_pool.tile([P, D4], BF16, tag="vT_ps")
                eT_ps = tps_pool.tile([P, D4], BF16, tag="eT_ps")
                for s_ in range(QUAD):  # sub-tile within quad
                    for h in range(2):  # 128-dim half
                        o = s_ * DIM + h * P
                        nc.tensor.transpose(
                            vT_ps[:, o : o + P], v_sb[:, c0 + o : c0 + o + P],
                            identity[:],
                        )
                        nc.tensor.transpose(
                            eT_ps[:, o : o + P], e_sb[:, c0 + o : c0 + o + P],
                            identity[:],
                        )
                vT_sb = tsb_pool.tile([P, D4], BF16, tag="vT_sb")
                eT_sb = tsb_pool.tile([P, D4], BF16, tag="eT_sb")
                nc.scalar.copy(vT_sb[:], vT_ps[:])
                nc.vector.tensor_copy(eT_sb[:], eT_ps[:])
                return (v_sb, e_sb, vo_sb, eo_sb, c0, vT_sb, eT_sb)

            def emit_quad_tail(p):
                """dots + s-copy + elementwise + fused adds for a quad."""
                v_sb, e_sb, vo_sb, eo_sb, c0, vT_sb, eT_sb = p
                s_ps = sps_pool.tile([P, QUAD * 8], F32, tag="s_ps")
                # per sub-tile s layout: cols 0-3 = v.(w_vv,w_ev,w_ve,w_ee),
                # cols 4-7 = e.(...); sub-tile s_ at col offset 8*s_
                for s_ in range(QUAD):
                    o = s_ * DIM
                    so = s_ * 8
                    nc.tensor.matmul(
                        s_ps[:, so : so + 4], vT_sb[:, o : o + P],
                        w4_sb[:, 0, :], start=True, stop=False,
                    )
                    nc.tensor.matmul(
                        s_ps[:, so : so + 4], vT_sb[:, o + P : o + DIM],
                        w4_sb[:, 1, :], start=False, stop=True,
                    )
                    nc.tensor.matmul(
                        s_ps[:, so + 4 : so + 8], eT_sb[:, o : o + P],
                        w4_sb[:, 0, :], start=True, stop=False,
                    )
                    nc.tensor.matmul(
                        s_ps[:, so + 4 : so + 8], eT_sb[:, o + P : o + DIM],
                        w4_sb[:, 1, :], start=False, stop=True,
                    )
                s_sb = ssb_pool.tile([P, QUAD * 8], F32)
                nc.scalar.copy(s_sb[:], s_ps[:])

                t_v = ut_pool.tile([P, D4], BF16, tag="t_v")
                t_e = ut_pool.tile([P, D4], BF16, tag="t_e")
                u_v = ut_pool.tile([P, D4], BF16, tag="u_v")
                u_e = ut_pool.tile([P, D4], BF16, tag="u_e")
                for s_ in range(QUAD):
                    o = s_ * DIM
                    so = s_ * 8
                    v_sub = v_sb[:, c0 + o : c0 + o + DIM]
                    e_sub = e_sb[:, c0 + o : c0 + o + DIM]
                    s_evv = s_sb[:, so + 4 : so + 5]  # e.w_vv
                    s_vev = s_sb[:, so + 1 : so + 2]  # v.w_ev
                    s_vee = s_sb[:, so + 3 : so + 4]  # v.w_ee
                    # t_v = e*(v.w_ev) + c1   (ACT, fused scale+bias)
                    nc.scalar.activation(
                        t_v[:, o : o + DIM], e_sub, ActFn.Identity,
                        bias=bias_sb[:, 0:1], scale=s_vev,
                    )
                    # t_e = e*(v.w_ee) + c2   (DVE tensor_scalar)
                    nc.vector.tensor_scalar(
                        t_e[:, o : o + DIM], e_sub, s_vee, bias_sb[:, 1:2],
                        AluOp.mult, AluOp.add,
                    )
                # u_v = v*(e.w_vv), u_e = v*(e.w_ve): one Pool pass per quad
                # each, s broadcast along free within each sub-tile block
                v_quad = v_sb[:, c0 : c0 + D4].rearrange("p (q d) -> p q d", q=QUAD)
                s_q = s_sb[:].rearrange("p (q c) -> p q c", q=QUAD)
                nc.gpsimd.tensor_tensor(
                    u_v[:].rearrange("p (q d) -> p q d", q=QUAD),
                    v_quad,
                    s_q[:, :, 4:5].broadcast_to([P, QUAD, DIM]),
                    AluOp.mult,
                )
                nc.gpsimd.tensor_tensor(
                    u_e[:].rearrange("p (q d) -> p q d", q=QUAD),
                    v_quad,
                    s_q[:, :, 6:7].broadcast_to([P, QUAD, DIM]),
                    AluOp.mult,
                )
                # fused quad adds (DVE tensor_tensor, 2x)
                nc.vector.tensor_tensor(
                    vo_sb[:, c0 : c0 + D4], u_v[:], t_v[:], AluOp.add
                )
                nc.vector.tensor_tensor(
                    eo_sb[:, c0 : c0 + D4], u_e[:], t_e[:], AluOp.add
                )

            FREE = ROWS_PER_PART * DIM  # 4096
            for t in range(N_MEGA):
                v_sb = vin_pool.tile([P, FREE], BF16)
                e_sb = ein_pool.tile([P, FREE], BF16)
                r0 = t * MEGA_ROWS
                nc.sync.dma_start(
                    out=v_sb[:],
                    in_=v_d[r0 : r0 + MEGA_ROWS, :].rearrange(
                        "(p g) d -> p (g d)", p=P
                    ),
                )
                nc.sync.dma_start(
                    out=e_sb[:],
                    in_=e_d[r0 : r0 + MEGA_ROWS, :].rearrange(
                        "(p g) d -> p (g d)", p=P
                    ),
                )
                vo_sb = vo_pool.tile([P, FREE], BF16)
                eo_sb = eo_pool.tile([P, FREE], BF16)

                for q in range(N_QUADS):
                    head = emit_quad_head(v_sb, e_sb, vo_sb, eo_sb, q)
                    if pend is not None:
                        emit_quad_tail(pend)
                    pend = head

                out_tiles.append((t, vo_sb, eo_sb))
                if len(out_tiles) > 1:
                    tt_, vo_prev, eo_prev = out_tiles.pop(0)
                    rr = tt_ * MEGA_ROWS
                    nc.scalar.dma_start(
                        out=vout_d[rr : rr + MEGA_ROWS, :].rearrange(
                            "(p g) d -> p (g d)", p=P
                        ),
                        in_=vo_prev[:],
                    )
                    nc.scalar.dma_start(
                        out=eout_d[rr : rr + MEGA_ROWS, :].rearrange(
                            "(p g) d -> p (g d)", p=P
                        ),
                        in_=eo_prev[:],
                    )

            emit_quad_tail(pend)
            for tt_, vo_prev, eo_prev in out_tiles:
                rr = tt_ * MEGA_ROWS
                nc.scalar.dma_start(
                    out=vout_d[rr : rr + MEGA_ROWS, :].rearrange(
                        "(p g) d -> p (g d)", p=P
                    ),
                    in_=vo_prev[:],
                )
                nc.scalar.dma_start(
                    out=eout_d[rr : rr + MEGA_ROWS, :].rearrange(
                        "(p g) d -> p (g d)", p=P
                    ),
                    in_=eo_prev[:],
                )

    nc.finalize()
    return nc


def _get_program():
    if "nc" not in _COMPILED:
        _COMPILED["nc"] = build_program()
    return _COMPILED["nc"]


def run(v, e, w_vv, b_vv, w_ev, b_ev, w_ve, b_ve, w_ee, b_ee, trace=False, **kw):
    import ml_dtypes

    BF = ml_dtypes.bfloat16
    nc = _get_program()

    w4 = np.stack(
        [
            np.asarray(w_vv, np.float32),
            np.asarray(w_ev, np.float32),
            np.asarray(w_ve, np.float32),
            np.asarray(w_ee, np.float32),
        ],
        axis=1,
    ).astype(BF)  # [256, 4]
    ident = np.eye(P, dtype=np.float32).astype(BF)
    bias = np.empty((P, 2), np.float32)
    bias[:, 0] = np.float32(b_vv) + np.float32(b_ev)
    bias[:, 1] = np.float32(b_ve) + np.float32(b_ee)

    v = np.asarray(v, np.float32).astype(BF)
    e = np.asarray(e, np.float32).astype(BF)
    in_maps = []
    for i in range(N_CORES):
        sl = slice(i * B_CORE, (i + 1) * B_CORE)
        in_maps.append(
            {"v": v[sl], "e": e[sl], "w4": w4, "ident": ident, "bias": bias}
        )

    res = run_bass_kernel_spmd(nc, in_maps, list(range(N_CORES)), trace=trace, **kw)
    v_out = np.concatenate(
        [np.asarray(r["vout"]).astype(np.float32) for r in res.results], axis=0
    )
    e_out = np.concatenate(
        [np.asarray(r["eout"]).astype(np.float32) for r in res.results], axis=0
    )
    return (v_out, e_out), res


def kernel(**inputs):
    (v_out, e_out), _ = run(**inputs)
    return (v_out, e_out)


if __name__ == "__main__":
    rng = np.random.default_rng(0)
    inputs = {
        "v": rng.standard_normal((B_FULL, DIM), dtype=np.float32),
        "e": rng.standard_normal((B_FULL, DIM), dtype=np.float32),
        "w_vv": rng.uniform(-0.0625, 0.0625, DIM).astype(np.float32),
        "b_vv": np.float32(0.01),
        "w_ev": rng.uniform(-0.0625, 0.0625, DIM).astype(np.float32),
        "b_ev": np.float32(-0.02),
        "w_ve": rng.uniform(-0.0625, 0.0625, DIM).astype(np.float32),
        "b_ve": np.float32(0.03),
        "w_ee": rng.uniform(-0.0625, 0.0625, DIM).astype(np.float32),
        "b_ee": np.float32(0.005),
    }
    v_out, e_out = kernel(**inputs)
    s1 = inputs["e"] @ inputs["w_vv"]
    s2 = inputs["v"] @ inputs["w_ev"]
    ref_v = inputs["v"] * s1[:, None] + inputs["e"] * s2[:, None] + (
        inputs["b_vv"] + inputs["b_ev"]
    )
    err = np.abs(v_out - ref_v).max() / np.abs(ref_v).max()
    print("smoke rel err v_out:", err)


# revision 10
# speedup vs baseline: 1.2962x; 1.2962x over previous
"""CrossCompressUnit kernel for TRN2, 8 NeuronCores, batch-sharded data parallel.

Math (per row b):
  v_out[b,:] = v[b,:]*(e[b].w_vv) + e[b,:]*(v[b].w_ev) + (b_vv+b_ev)
  e_out[b,:] = v[b,:]*(e[b].w_ve) + e[b,:]*(v[b].w_ee) + (b_ve+b_ee)

v4 design (bf16 end-to-end; DMA floor ~93us/core at 33.6MB traffic):
  - Host converts v,e to bf16; device reads/writes bf16 HBM; host upcasts
    outputs to f32 (rel-err ~7e-3 << 2e-2 gate).
  - All row-dots on PE: per [128,256] sub-tile, 4 transposes (bf16 LDW+MM
    with FWL) + 4 dot matmuls (vT/eT halves x [128,(2,4)] w4, f32 PSUM
    accumulation over the two 128-dim halves).
  - Work fused at QUAD granularity (4 sub-tiles, [128,1024]) to amortize
    the ~180-280ns/instr DVE/ACT overheads: one PSUM->SBUF copy per input
    per quad (ACT: vT, DVE: eT), one [128,32] s-copy (ACT), one final add
    per output per quad (DVE tensor_tensor, 2x), one Pool broadcast
    multiply per quad for u_e.
  - All DVE operands bf16 (incl. s scalars and bias) to enable the DVE
    2x/4x perf modes, which hardware only grants when every operand is
    2-byte.
  - Loads on sync (HWDGE SP ring), stores on scalar (HWDGE ACT ring),
    1MB DMAs ([128,4096] bf16 mega-tiles).
"""

import sys

sys.path.insert(0, "/opt/trn_rl_repo")

import numpy as np

import concourse.bass as bass
import concourse.bacc as bacc_mod
import concourse.mybir as mybir
from concourse.bass_utils import run_bass_kernel_spmd
from concourse.tile import TileContext

N_CORES = 8
B_FULL = 131072
DIM = 256
B_CORE = B_FULL // N_CORES  # 16384
P = 128

MEGA_ROWS = 2048                  # rows per mega-tile -> [128,4096] bf16 = 1MB DMA
ROWS_PER_PART = MEGA_ROWS // P    # 16 sub-tiles per mega-tile
N_MEGA = B_CORE // MEGA_ROWS      # 8
QUAD = 4                          # sub-tiles fused per quad unit
N_QUADS = ROWS_PER_PART // QUAD   # 4 quads per mega-tile

F32 = mybir.dt.float32
BF16 = mybir.dt.bfloat16
AluOp = mybir.AluOpType
ActFn = mybir.ActivationFunctionType

_COMPILED = {}


def build_program():
    nc = bacc_mod.Bacc()

    v_d = nc.declare_dram_parameter("v", [B_CORE, DIM], BF16, isOutput=False)
    e_d = nc.declare_dram_parameter("e", [B_CORE, DIM], BF16, isOutput=False)
    w4_d = nc.declare_dram_parameter("w4", [DIM, 4], BF16, isOutput=False)
    ident_d = nc.declare_dram_parameter("ident", [P, P], BF16, isOutput=False)
    bias_d = nc.declare_dram_parameter("bias", [P, 2], F32, isOutput=False)
    vout_d = nc.declare_dram_parameter("vout", [B_CORE, DIM], BF16, isOutput=True)
    eout_d = nc.declare_dram_parameter("eout", [B_CORE, DIM], BF16, isOutput=True)

    D4 = QUAD * DIM  # 1024, one quad

    with TileContext(nc) as tc:
        with (
            tc.tile_pool(name="consts", bufs=1) as consts,
            tc.tile_pool(name="vin", bufs=3) as vin_pool,
            tc.tile_pool(name="ein", bufs=3) as ein_pool,
            tc.tile_pool(name="vo", bufs=3) as vo_pool,
            tc.tile_pool(name="eo", bufs=3) as eo_pool,
            tc.tile_pool(name="tsb", bufs=2) as tsb_pool,
            tc.tile_pool(name="ssb", bufs=3) as ssb_pool,
            tc.tile_pool(name="ut", bufs=2) as ut_pool,
            tc.tile_pool(name="tps", bufs=2, space=bass.MemorySpace.PSUM) as tps_pool,
            tc.tile_pool(name="sps", bufs=2, space=bass.MemorySpace.PSUM) as sps_pool,
        ):
            # --- constants ---
            identity = consts.tile([P, P], BF16)
            nc.sync.dma_start(out=identity[:], in_=ident_d[:])
            w4_sb = consts.tile([P, 2, 4], BF16)  # [dim_in_half, half, 4]
            nc.sync.dma_start(
                out=w4_sb[:], in_=w4_d.rearrange("(h p) w -> p h w", h=2)
            )
            bias_sb = consts.tile([P, 2], F32)
            nc.sync.dma_start(out=bias_sb[:], in_=bias_d[:])

            out_tiles = []
            pend = None  # quad awaiting dots+elementwise (1-quad software skew)

            def emit_quad_head(v_sb, e_sb, vo_sb, eo_sb, q):
                """transposes + PSUM->SBUF copies for quad q of this mega."""
                c0 = q * D4
                vT_ps = tps_pool.tile([P, D4], BF16, tag="vT_ps")
                eT_ps = tps_pool.tile([P, D4], BF16, tag="eT_ps")
                for s_ in range(QUAD):  # sub-tile within quad
                    for h in range(2):  # 128-dim half
                        o = s_ * DIM + h * P
                        nc.tensor.transpose(
                            vT_ps[:, o : o + P], v_sb[:, c0 + o : c0 + o + P],
                            identity[:],
                        )
                        nc.tensor.transpose(
                            eT_ps[:, o : o + P], e_sb[:, c0 + o : c0 + o + P],
                            identity[:],
                        )
                vT_sb = tsb_pool.tile([P, D4], BF16, tag="vT_sb")
                eT_sb = tsb_pool.tile([P, D4], BF16, tag="eT_sb")
                nc.scalar.copy(vT_sb[:], vT_ps[:])
                nc.vector.tensor_copy(eT_sb[:], eT_ps[:])
                return (v_sb, e_sb, vo_sb, eo_sb, c0, vT_sb, eT_sb)

            def emit_quad_tail(p):
                """dots + s-copy + elementwise + fused adds for a quad."""
                v_sb, e_sb, vo_sb, eo_sb, c0, vT_sb, eT_sb = p
                s_ps = sps_pool.tile([P, QUAD * 8], F32, tag="s_ps")
                # per sub-tile s layout: cols 0-3 = v.(w_vv,w_ev,w_ve,w_ee),
                # cols 4-7 = e.(...); sub-tile s_ at col offset 8*s_
                for s_ in range(QUAD):
                    o = s_ * DIM
                    so = s_ * 8
                    nc.tensor.matmul(
                        s_ps[:, so : so + 4], vT_sb[:, o : o + P],
                        w4_sb[:, 0, :], start=True, stop=False,
                    )
                    nc.tensor.matmul(
                        s_ps[:, so : so + 4], vT_sb[:, o + P : o + DIM],
                        w4_sb[:, 1, :], start=False, stop=True,
                    )
                    nc.tensor.matmul(
                        s_ps[:, so + 4 : so + 8], eT_sb[:, o : o + P],
                        w4_sb[:, 0, :], start=True, stop=False,
                    )
                    nc.tensor.matmul(
                        s_ps[:, so + 4 : so + 8], eT_sb[:, o + P : o + DIM],
                        w4_sb[:, 1, :], start=False, stop=True,
                    )
                s_sb = ssb_pool.tile([P, QUAD * 8], F32)
                nc.scalar.copy(s_sb[:], s_ps[:])

                t_v = ut_pool.tile([P, D4], BF16, tag="t_v")
                t_e = ut_pool.tile([P, D4], BF16, tag="t_e")
                u_v = ut_pool.tile([P, D4], BF16, tag="u_v")
                u_e = ut_pool.tile([P, D4], BF16, tag="u_e")
                for s_ in range(QUAD):
                    o = s_ * DIM
                    so = s_ * 8
                    v_sub = v_sb[:, c0 + o : c0 + o + DIM]
                    e_sub = e_sb[:, c0 + o : c0 + o + DIM]
                    s_evv = s_sb[:, so + 4 : so + 5]  # e.w_vv
                    s_vev = s_sb[:, so + 1 : so + 2]  # v.w_ev
                    s_vee = s_sb[:, so + 3 : so + 4]  # v.w_ee
                    # t_v = e*(v.w_ev) + c1   (ACT, fused scale+bias)
                    nc.scalar.activation(
                        t_v[:, o : o + DIM], e_sub, ActFn.Identity,
                        bias=bias_sb[:, 0:1], scale=s_vev,
                    )
                    # t_e = e*(v.w_ee) + c2   (DVE tensor_scalar)
                    nc.vector.tensor_scalar(
                        t_e[:, o : o + DIM], e_sub, s_vee, bias_sb[:, 1:2],
                        AluOp.mult, AluOp.add,
                    )
                # u_v = v*(e.w_vv): alternate DVE ts / ACT scale by sub-tile;
                # u_e = v*(e.w_ve): Pool tensor_tensor with broadcast s
                for s_ in range(QUAD):
                    o = s_ * DIM
                    so = s_ * 8
                    v_sub = v_sb[:, c0 + o : c0 + o + DIM]
                    s_evv = s_sb[:, so + 4 : so + 5]
                    s_eve = s_sb[:, so + 6 : so + 7]
                    if s_ % 2 == 0:
                        nc.vector.tensor_scalar(
                            u_v[:, o : o + DIM], v_sub, s_evv, None, AluOp.mult
                        )
                    else:
                        nc.scalar.activation(
                            u_v[:, o : o + DIM], v_sub, ActFn.Copy,
                            bias=0.0, scale=s_evv,
                        )
                    nc.gpsimd.tensor_tensor(
                        u_e[:, o : o + DIM], v_sub,
                        s_eve.broadcast_to([P, DIM]), AluOp.mult,
                    )
                # fused quad adds (DVE tensor_tensor, 2x)
                nc.vector.tensor_tensor(
                    vo_sb[:, c0 : c0 + D4], u_v[:], t_v[:], AluOp.add
                )
                nc.vector.tensor_tensor(
                    eo_sb[:, c0 : c0 + D4], u_e[:], t_e[:], AluOp.add
                )

            FREE = ROWS_PER_PART * DIM  # 4096
            for t in range(N_MEGA):
                v_sb = vin_pool.tile([P, FREE], BF16)
                e_sb = ein_pool.tile([P, FREE], BF16)
                r0 = t * MEGA_ROWS
                nc.sync.dma_start(
                    out=v_sb[:],
                    in_=v_d[r0 : r0 + MEGA_ROWS, :].rearrange(
                        "(p g) d -> p (g d)", p=P
                    ),
                )
                nc.sync.dma_start(
                    out=e_sb[:],
                    in_=e_d[r0 : r0 + MEGA_ROWS, :].rearrange(
                        "(p g) d -> p (g d)", p=P
                    ),
                )
                vo_sb = vo_pool.tile([P, FREE], BF16)
                eo_sb = eo_pool.tile([P, FREE], BF16)

                for q in range(N_QUADS):
                    head = emit_quad_head(v_sb, e_sb, vo_sb, eo_sb, q)
                    if pend is not None:
                        emit_quad_tail(pend)
                    pend = head

                out_tiles.append((t, vo_sb, eo_sb))
                if len(out_tiles) > 1:
                    tt_, vo_prev, eo_prev = out_tiles.pop(0)
                    rr = tt_ * MEGA_ROWS
                    nc.scalar.dma_start(
                        out=vout_d[rr : rr + MEGA_ROWS, :].rearrange(
                            "(p g) d -> p (g d)", p=P
                        ),
                        in_=vo_prev[:],
                    )
                    nc.scalar.dma_start(
                        out=eout_d[rr : rr + MEGA_ROWS, :].rearrange(
                            "(p g) d -> p (g d)", p=P
                        ),
                        in_=eo_prev[:],
                    )

            emit_quad_tail(pend)
            for tt_, vo_prev, eo_prev in out_tiles:
                rr = tt_ * MEGA_ROWS
                nc.scalar.dma_start(
                    out=vout_d[rr : rr + MEGA_ROWS, :].rearrange(
                        "(p g) d -> p (g d)", p=P
                    ),
                    in_=vo_prev[:],
                )
                nc.scalar.dma_start(
                    out=eout_d[rr : rr + MEGA_ROWS, :].rearrange(
                        "(p g) d -> p (g d)", p=P
                    ),
                    in_=eo_prev[:],
                )

    nc.finalize()
    return nc


def _get_program():
    if "nc" not in _COMPILED:
        _COMPILED["nc"] = build_program()
    return _COMPILED["nc"]


def run(v, e, w_vv, b_vv, w_ev, b_ev, w_ve, b_ve, w_ee, b_ee, trace=False, **kw):
    import ml_dtypes

    BF = ml_dtypes.bfloat16
    nc = _get_program()

    w4 = np.stack(
        [
            np.asarray(w_vv, np.float32),
            np.asarray(w_ev, np.float32),
            np.asarray(w_ve, np.float32),
            np.asarray(w_ee, np.float32),
        ],
        axis=1,
    ).astype(BF)  # [256, 4]
    ident = np.eye(P, dtype=np.float32).astype(BF)
    bias = np.empty((P, 2), np.float32)
    bias[:, 0] = np.float32(b_vv) + np.float32(b_ev)
    bias[:, 1] = np.float32(b_ve) + np.float32(b_ee)

    v = np.asarray(v, np.float32).astype(BF)
    e = np.asarray(e, np.float32).astype(BF)
    in_maps = []
    for i in range(N_CORES):
        sl = slice(i * B_CORE, (i + 1) * B_CORE)
        in_maps.append(
            {"v": v[sl], "e": e[sl], "w4": w4, "ident": ident, "bias": bias}
        )

    res = run_bass_kernel_spmd(nc, in_maps, list(range(N_CORES)), trace=trace, **kw)
    v_out = np.concatenate(
        [np.asarray(r["vout"]).astype(np.float32) for r in res.results], axis=0
    )
    e_out = np.concatenate(
        [np.asarray(r["eout"]).astype(np.float32) for r in res.results], axis=0
    )
    return (v_out, e_out), res


def kernel(**inputs):
    (v_out, e_out), _ = run(**inputs)
    return (v_out, e_out)


if __name__ == "__main__":
    rng = np.random.default_rng(0)
    inputs = {
        "v": rng.standard_normal((B_FULL, DIM), dtype=np.float32),
        "e": rng.standard_normal((B_FULL, DIM), dtype=np.float32),
        "w_vv": rng.uniform(-0.0625, 0.0625, DIM).astype(np.float32),
        "b_vv": np.float32(0.01),
        "w_ev": rng.uniform(-0.0625, 0.0625, DIM).astype(np.float32),
        "b_ev": np.float32(-0.02),
        "w_ve": rng.uniform(-0.0625, 0.0625, DIM).astype(np.float32),
        "b_ve": np.float32(0.03),
        "w_ee": rng.uniform(-0.0625, 0.0625, DIM).astype(np.float32),
        "b_ee": np.float32(0.005),
    }
    v_out, e_out = kernel(**inputs)
    s1 = inputs["e"] @ inputs["w_vv"]
    s2 = inputs["v"] @ inputs["w_ev"]
    ref_v = inputs["v"] * s1[:, None] + inputs["e"] * s2[:, None] + (
        inputs["b_vv"] + inputs["b_ev"]
    )
    err = np.abs(v_out - ref_v).max() / np.abs(ref_v).max()
    print("smoke rel err v_out:", err)
